# revision 96
# baseline (speedup 1.0000x reference)
"""Causal multi-head attention (B=4, S=2048, E=768, N=12 heads, H=64) on 8
Trainium2 NeuronCores.

Sharding: core c handles batch c//2 and heads (c%2)*6 .. +6 (tensor parallel
over heads within a batch pair). No collectives: each core emits a partial
out^T = (sum over its 6 heads of z @ W_O) + b_O/2, and the host sums the two
partials per batch and transposes back.

Layout: all device math runs in a transposed layout (seq on the free axis):
  xT [E, S] per batch (host-transposed)
  Q^T/K^T per head-pair  [128 (2x64h), S] in per-512-column tiles
  V natural [S, 65*6]  (65th column per head is all-ones -> PV matmul row 64
                        accumulates the softmax denominator for free)
  S^T [k, q] scores, both heads of a pair computed concurrently in the PE
  array via tile_position row groups; P = exp(scale*S^T), diagonal blocks
  multiplied by a 0/1 keep-mask; z^T [64, q] normalized by 1/denominator
  (fast DVE reciprocal + gpsimd partition_broadcast);
  out^T [E, S] accumulated over head pairs (K=128 contraction).

Scheduling: projection blocks for query block qb+1 and the output projection
for qb are emitted as single-instruction closures drained into attention(qb+1)
iterations, filling PE bubbles left by the ACT-bound exp pipeline.

Perf notes (this revision, ~200us vs 230us predecessor):
- HAM warm-up: ~24 junk matmuls into the idle 'z' PSUM banks flip the PE
  clock gate from 1.2 to 2.4 GHz before the first DMA-gated real matmul.
- Causal band trim: diagonal-band blocks (kb >= 4qb) compute only q-columns
  >= (kb-4qb)*128 in QK / exp (strided [128,2,N] AP) / PV; the mask multiply
  shrinks to the [128, 2x128] diagonal chunk.
- Engine rebalance: K/Q bias copies and out-proj bias adds moved ACT -> DVE;
  softmax reciprocal+broadcast run immediately after each pair's PV (only the
  normalize multiply is deferred); final-quarter stores split across both
  HWDGE rings.
- Quarter balance: attention(0..2) are PE-drain-saturated while attention(3)
  is ACT-bound with idle PE, so quarter 3's K/Q pair-1/2 and V chains drain
  into attention(3) itself, gated by emission-order markers (prereq per pair,
  vt_gate before the first band PV). Pushing more than this into (1)/(2)
  measured worse — they have no spare bubble capacity.
- xT arrives in 8 batched [128, 3*512] DMAs (descriptor issue on the sync
  engine costs ~600ns each; 24 issues serialized the early phase); wv and the
  late-needed wo/bo ride the scalar ring (wo appended AFTER the critical
  wk/wq — interleaving them measured +40us), bv/masks the sync ring right
  after xT quarter 0; gpsimd touches no SWDGE queue at all (it moves only
  ~80GB/s, starved the first V chains, and its exit dge_drain shrinks when
  unused); tiny bias/mask tensors are packed into single descriptors;
  outputs store as bf16 (halves 6MB of store traffic; rel err 4.1e-3 ->
  4.4e-3, gate 2e-2).
Pitfalls learned on HW: PSUM reads need 32-aligned partition bases; custom
DVE ops and gpsimd partition_broadcast cannot partition-shift; DMA cannot
read PSUM; a gpsimd tensor op amid SWDGE traffic forces a ~16us dge_drain.
"""

import sys

sys.path.insert(0, "/opt/trn_rl_repo")

import numpy as np

B, S, E = 4, 2048, 768
N_HEADS, H = 12, 64
HPC = 6           # heads per core
PAIRS = 3         # head pairs per core
EC = E // 128     # 6 e-chunks
QB = 512          # query block (free dim of most matmuls)
NQB = S // QB     # 4
KB = 128          # key sub-block (partition dim of S^T)
SC = S // 128     # 16 s-chunks for V
VW = 65           # V width per head incl. ones column
SCALE = 1.0 / np.sqrt(np.float32(H))

# Compute dtype for PE-facing tensors: "float32r" streams fp32 at full PE rate
# when the moving dim >= 256; "float32" is exact but 4 cycles/row; "bfloat16"
# halves SBUF footprint.
COMPUTE_DT = "bfloat16"

_g = {"nc": None}


def _np_dt():
    if COMPUTE_DT == "bfloat16":
        import ml_dtypes

        return ml_dtypes.bfloat16
    return np.float32


def _build():
    from concourse import bacc, tile, mybir

    F32 = mybir.dt.float32
    DT = getattr(mybir.dt, COMPUTE_DT)
    def R(ap):
        return ap

    nc = bacc.Bacc("TRN2", target_bir_lowering=False, debug=False, num_devices=8)

    d_xT = nc.dram_tensor("xT", [E, S], DT, kind="ExternalInput").ap()
    d_wq = nc.dram_tensor("wq", [PAIRS * 128, E], DT, kind="ExternalInput").ap()
    d_wk = nc.dram_tensor("wk", [PAIRS * 128, E], DT, kind="ExternalInput").ap()
    d_wv = nc.dram_tensor("wv", [128, VW * HPC * EC], DT, kind="ExternalInput").ap()
    d_wo = nc.dram_tensor("wo", [PAIRS * 128, E], DT, kind="ExternalInput").ap()
    # bq|bk packed as columns; bo chunks as columns: tiny per-tensor DMA
    # descriptors cost ~600ns of engine issue time each
    d_bqk = nc.dram_tensor("bqk", [128, 2 * PAIRS], F32, kind="ExternalInput").ap()
    d_bv = nc.dram_tensor("bv", [128, VW * HPC], F32, kind="ExternalInput").ap()
    d_bo = nc.dram_tensor("bo", [128, EC], F32, kind="ExternalInput").ap()
    d_mask = nc.dram_tensor("mask", [KB, 4 * 2 * KB], DT, kind="ExternalInput").ap()
    d_iden = nc.dram_tensor("iden", [128, 128], DT, kind="ExternalInput").ap()
    d_out = nc.dram_tensor("outT", [E, S], DT, kind="ExternalOutput").ap()

    Exp = mybir.ActivationFunctionType.Exp
    Copy = mybir.ActivationFunctionType.Copy

    with tile.TileContext(nc) as tc:
        with tc.tile_pool(name="persist", bufs=1) as pp, \
             tc.tile_pool(name="work", bufs=4) as wp, \
             tc.tile_pool(name="zsb", bufs=3) as zp, \
             tc.tile_pool(name="outsb", bufs=4) as op, \
             tc.tile_pool(name="psA", bufs=1, space="PSUM") as psA:

            # ---- HAM warm-up --------------------------------------------------
            # The PE clock gate (HAM) starts at K=4/8 (1.2 GHz) and only
            # promotes to 8/8 after ~3.4us of sustained PE activity. Real work
            # can't start until the first DMAs land (~10.5us: ~6us engine
            # preamble + DGE wake + transfer), so without a primer the whole
            # first attention block runs at half clock. Issue junk matmuls on
            # a memset tile to flip the HAM before real work arrives.
            warm = pp.tile([128, QB], DT, tag="warm", name="warm")
            nc.gpsimd.memset(warm[:], 0.0)
            # junk targets the 'z' psum banks: those are first needed by
            # attention(0)'s PV (~18us), so the warm-up never blocks the
            # first projection chains (which use the 'misc' banks)
            wps = [psA.tile([VW, QB], F32, tag="z", bufs=2, name=f"warmps{i}")
                   for i in range(2)]
            for i in range(24):
                nc.tensor.matmul(wps[i % 2][:], warm[:, 0:VW], warm[:],
                                 start=True, stop=True)

            # ---- static tiles -------------------------------------------------
            # DMA routing: weights for the first projections go on the ACT
            # HWDGE ring, xT halves on the SP ring (the two rings run in
            # parallel), and everything not needed until later (W_O, b_O,
            # masks, V weights/biases) on the gpsimd SWDGE queues.
            wq, wk, wo = [], [], []
            for p in range(PAIRS):
                tk = pp.tile([128, E], DT, tag=f"wk{p}", name=f"wk{p}")
                nc.scalar.dma_start(tk[:], d_wk[p * 128:(p + 1) * 128, :])
                wk.append(tk)
                tq = pp.tile([128, E], DT, tag=f"wq{p}", name=f"wq{p}")
                nc.scalar.dma_start(tq[:], d_wq[p * 128:(p + 1) * 128, :])
                wq.append(tq)
                to = pp.tile([128, E], DT, tag=f"wo{p}", name=f"wo{p}")
                wo.append(to)
            bqk = pp.tile([128, 2 * PAIRS], F32, tag="bqk", name="bqk")
            nc.scalar.dma_start(bqk[:], d_bqk[:, :])
            bq = [bqk[:, p:p + 1] for p in range(PAIRS)]
            bk = [bqk[:, PAIRS + p:PAIRS + p + 1] for p in range(PAIRS)]
            bo = []
            wv_all = pp.tile([128, VW * HPC * EC], DT, tag="wv", name="wv_all")
            nc.scalar.dma_start(wv_all[:], d_wv[:, :])
            wv = [wv_all[:, e * VW * HPC:(e + 1) * VW * HPC] for e in range(EC)]
            # xT in 8 batched transfers ([128, 3, 512] each): one dma_start
            # costs ~600ns of engine time, so 8 descriptors instead of 24
            # frees the sync engine ~10us earlier; half-quarter granularity
            # still lets the first projection chain start on e0-2.
            # sync-ring order matters (transfers run in order at wire speed):
            # xT quarter 0 first (first projections), then wv/bv/masks
            # (needed ~18-25us; the gpsimd SWDGE queue is ~80GB/s-slow and
            # the scalar ring's issue flow-control would hold them to ~18us),
            # then xT quarters 1-3 (needed at ~19/~40/~60us).
            d_xT3 = d_xT.rearrange("(e p) s -> p e s", p=128)
            xbig = [[None, None] for _ in range(4)]
            for quarter in range(4):
                for half in range(2):
                    xbig[quarter][half] = pp.tile(
                        [128, 3 * QB], DT, tag=f"xt{quarter}_{half}",
                        name=f"xt{quarter}_{half}")

            def _xtq_dma(quarter):
                hs = slice(quarter * QB, (quarter + 1) * QB)
                for half in range(2):
                    t = xbig[quarter][half]
                    nc.sync.dma_start(
                        t[:].rearrange("p (e s) -> p e s", s=QB),
                        d_xT3[:, 3 * half:3 * half + 3, hs])



            def xt(e, sb, c0=0, c1=QB):
                t = xbig[sb][e // 3]
                base = (e % 3) * QB
                return t[:, base + c0:base + c1]
            for p in range(PAIRS):
                nc.scalar.dma_start(wo[p][:], d_wo[p * 128:(p + 1) * 128, :])
            boall = pp.tile([128, EC], F32, tag="bo", name="boall")
            nc.scalar.dma_start(boall[:], d_bo[:, :])
            iden = pp.tile([128, 128], DT, tag="iden", name="iden")
            nc.scalar.dma_start(iden[:], d_iden[:, :])
            bo = [boall[:, e:e + 1] for e in range(EC)]
            bv = pp.tile([128, VW * HPC], F32, tag="bv")
            maskall = pp.tile([KB, 4 * 2 * KB], DT, tag="mask", name="maskall")
            masks = [maskall[:, o * 2 * KB:(o + 1) * 2 * KB] for o in range(4)]

            _xtq_dma(0)
            nc.sync.dma_start(bv[:], d_bv[:, :])
            nc.sync.dma_start(maskall[:], d_mask[:, :])
            for quarter in range(1, 4):
                _xtq_dma(quarter)

            kt = [[pp.tile([128, QB], DT, tag=f"kt{p}_{sb}", name=f"kt{p}_{sb}")
                   for sb in range(NQB)] for p in range(PAIRS)]
            qt = [[pp.tile([128, QB], DT, tag=f"qt{p}_{sb}", name=f"qt{p}_{sb}")
                   for sb in range(NQB)] for p in range(PAIRS)]
            vt = [pp.tile([128, VW * HPC], DT, tag=f"vt{s}", name=f"vt{s}") for s in range(SC)]

            Iden = mybir.ActivationFunctionType.Identity

            def _mk_chain():
                def chain(name, width, lhs_of_e, rhs_of_e, copy_out):
                    st = {}
                    def mk(e):
                        def step():
                            if e == 0:
                                st["ps"] = psA.tile(
                                    [128, width], F32, tag="misc", bufs=2,
                                    name=name)
                            nc.tensor.matmul(st["ps"][:],
                                             R(lhs_of_e(e)), R(rhs_of_e(e)),
                                             start=(e == 0), stop=(e == EC - 1))
                        return step
                    for e in range(EC):
                        yield mk(e)
                    yield lambda: copy_out(st["ps"])
                return chain

            def kq_pair_ops(sb, p, chain=None):
                # bias-add copies on DVE, keeping ACT free for the exp pipeline
                chain = chain or _mk_chain()
                kcopy = lambda ps, p=p, sb=sb: nc.vector.tensor_scalar_add(
                    kt[p][sb][:], ps[:], bk[p])
                qcopy = lambda ps, p=p, sb=sb: nc.vector.tensor_scalar_add(
                    qt[p][sb][:], ps[:], bq[p])
                yield from chain(
                    f"kps{p}_{sb}", QB,
                    lambda e, p=p: wk[p][:, e * 128:(e + 1) * 128],
                    lambda e, sb=sb: xt(e, sb), kcopy)
                yield from chain(
                    f"qps{p}_{sb}", QB,
                    lambda e, p=p: wq[p][:, e * 128:(e + 1) * 128],
                    lambda e, sb=sb: xt(e, sb), qcopy)

            def kq_ops(sb, chain=None):
                for p in range(PAIRS):
                    yield from kq_pair_ops(sb, p, chain)

            def v_ops(sb, chain=None):
                chain = chain or _mk_chain()
                for s in range(4 * sb, 4 * sb + 4):
                    yield from chain(
                        f"vps{s}", VW * HPC,
                        lambda e, sb=sb, s=s: xt(e, sb, (s % 4) * 128, (s % 4 + 1) * 128),
                        lambda e: wv[e],
                        lambda ps, s=s: nc.vector.tensor_add(
                            vt[s][:], ps[:], bv[:]))

            def make_normalize(qb, zpair):
                def normalize(head, zsb, bcast, unused=False):
                    # deferred: z * (1/denom), recip+broadcast already done.
                    # NB must stay on DVE: a gpsimd tensor op forces a ~16us
                    # dge_drain (SWDGE<->compute mode switch) on that engine.
                    p, sub = head // 2, head % 2
                    hsl = slice(sub * 64, sub * 64 + 64)
                    nc.vector.tensor_mul(zpair[p][hsl, :], zsb[:], bcast[:])
                return normalize

            def attention(qb, drain=None, late=None, last_pair_drain=None,
                          zpair_override=None, prereq=None, vt_gate=None):
                q0 = qb * QB
                qsl = slice(q0, q0 + QB)
                nkb = 4 * qb + 4
                # drain elements: zero-arg closures, or ("m", key) markers
                dq = list(drain) if drain is not None else []
                seen = set()
                iters = [PAIRS * max(nkb - 1, 1), 0]

                def _pop1():
                    el = dq.pop(0)
                    if isinstance(el, tuple):
                        seen.add(el[1])
                    else:
                        el()

                def drain_some():
                    if not dq:
                        return
                    n = max(1, -(-len(dq) // max(iters[0] - iters[1], 1)))
                    for _ in range(n):
                        if dq:
                            _pop1()
                    iters[1] += 1

                def drain_until(key):
                    # force-drain so a prerequisite chain is fully EMITTED
                    # before instructions that depend on it (emission order on
                    # an engine is execution order — a dep on a later
                    # instruction would deadlock)
                    while key not in seen and dq:
                        _pop1()
                zpair = zpair_override or [
                    zp.tile([128, QB], DT, tag=f"zp{p}", name=f"zp{p}_{qb}")
                    for p in range(PAIRS)]
                normalize = make_normalize(qb, zpair)
                pending = []
                for p in range(PAIRS):
                    if prereq and p in prereq:
                        drain_until(prereq[p])
                    zab = [psA.tile([VW, QB], F32, tag="z", bufs=2,
                                    name=f"zps{qb}_{2 * p + s}") for s in range(2)]

                    def qk(kb):
                        # both heads of the pair, concurrent via PE row groups.
                        # Diagonal-band blocks (kb >= 4qb) only need queries
                        # q >= (kb-4qb)*128: trim the streamed q range.
                        co = max(0, (kb - 4 * qb) * KB)
                        sps = psA.tile([KB, 2 * QB], F32, tag="s", bufs=2,
                                       name=f"sps{qb}_{p}_{kb}")
                        ktt = kt[p][kb // 4]
                        ksl = slice((kb % 4) * KB, (kb % 4 + 1) * KB)
                        nc.tensor.matmul(
                            sps[:, co:QB], R(ktt[0:64, ksl]), R(qt[p][qb][0:64, co:QB]),
                            start=True, stop=True, tile_position=(0, 0))
                        nc.tensor.matmul(
                            sps[:, QB + co:2 * QB], R(ktt[64:128, ksl]), R(qt[p][qb][64:128, co:QB]),
                            start=True, stop=True, tile_position=(64, 0))
                        return sps

                    def pv(kb, sps):
                        co = max(0, (kb - 4 * qb) * KB)
                        pt = wp.tile([KB, 2 * QB], DT, tag="p", bufs=6,
                                     name=f"pt{qb}_{p}_{kb}")
                        if co:
                            # strided [128, 2, QB-co] view covering both heads
                            sv = sps[:].rearrange("k (two q) -> k two q", two=2)[:, :, co:]
                            ptv = pt[:].rearrange("k (two q) -> k two q", two=2)[:, :, co:]
                            nc.scalar.activation(ptv, sv, Exp, scale=float(SCALE))
                        else:
                            nc.scalar.activation(pt[:], sps[:], Exp, scale=float(SCALE))
                        if kb >= 4 * qb:  # diagonal 128-col chunk: zero out k > q
                            o = kb - 4 * qb
                            ptd = pt[:].rearrange("k (two q) -> k two q", two=2)[:, :, co:co + KB]
                            mv = masks[o].rearrange("k (two q) -> k two q", two=2)
                            nc.vector.tensor_mul(ptd, ptd, mv)
                        for s in range(2):
                            nc.tensor.matmul(
                                zab[s][:, co:QB], R(vt[kb][:, (2 * p + s) * VW:(2 * p + s + 1) * VW]),
                                R(pt[:, s * QB + co:(s + 1) * QB]),
                                start=(kb == 0), stop=(kb == nkb - 1))

                    prev = qk(0)
                    for kb in range(1, nkb):
                        cur = qk(kb)
                        if vt_gate and kb - 1 == 4 * qb:
                            # band PV needs this quarter's vt chains emitted
                            drain_until(vt_gate)
                        pv(kb - 1, prev)
                        drain_some()
                        prev = cur
                        if kb == 2:
                            for args in pending:
                                normalize(*args)
                            pending = []
                            if p == PAIRS - 1 and last_pair_drain is not None:
                                dq.extend(last_pair_drain)
                    pv(nkb - 1, prev)
                    drain_some()

                    last = (qb == NQB - 1 and p == PAIRS - 1)
                    if last:
                        # tail: spread the readout chain across engines so the
                        # recip -> bcast -> mul critical path starts as early
                        # as possible (pass2 waits on the muls)
                        for s in range(2):
                            head = 2 * p + s
                            den = wp.tile([1, QB], F32, tag="den",
                                          name=f"den{qb}_{head}")
                            nc.scalar.activation(den[:], zab[s][64:65, :], Iden)
                            recipf = wp.tile([1, QB], F32, tag="recipf",
                                             name=f"recipf{qb}_{head}")
                            nc.vector.reciprocal_approx_fast(recipf[:], den[:])
                            zsb = wp.tile([64, QB], F32, tag="zc",
                                          name=f"zsb{qb}_{head}")
                            if s == 0:
                                nc.scalar.activation(zsb[:], zab[s][0:64, :], Iden)
                            else:
                                nc.vector.tensor_copy(zsb[:], zab[s][0:64, :])
                            bcast = wp.tile([64, QB], F32, tag="bcast",
                                            name=f"bcast{qb}_{head}")
                            nc.gpsimd.partition_broadcast(bcast[:], recipf[:])
                            pending.append((head, zsb, bcast, s == 0))
                    else:
                        # PSUM-freeing copies first (the next pair's PV waits
                        # on the zab banks), recip/broadcast after; in the
                        # PE-bound early quarters head 1's copies go to the
                        # then-idle ACT so the banks free ~2x sooner
                        zts = []
                        for s in range(2):
                            head = 2 * p + s
                            den = wp.tile([1, QB], F32, tag="den",
                                          name=f"den{qb}_{head}")
                            zsb = wp.tile([64, QB], F32, tag="zc",
                                          name=f"zsb{qb}_{head}")
                            if qb <= 1 and s == 1:
                                nc.scalar.activation(den[:], zab[s][64:65, :], Iden)
                                nc.scalar.activation(zsb[:], zab[s][0:64, :], Iden)
                            else:
                                nc.vector.tensor_copy(den[:], zab[s][64:65, :])
                                nc.vector.tensor_copy(zsb[:], zab[s][0:64, :])
                            zts.append((head, den, zsb))
                        for head, den, zsb in zts:
                            recipf = wp.tile([1, QB], F32, tag="recipf",
                                             name=f"recipf{qb}_{head}")
                            nc.vector.reciprocal_approx_fast(recipf[:], den[:])
                            bcast = wp.tile([64, QB], F32, tag="bcast",
                                            name=f"bcast{qb}_{head}")
                            nc.gpsimd.partition_broadcast(bcast[:], recipf[:])
                            pending.append((head, zsb, bcast, False))
                for args in pending:
                    normalize(*args)
                while dq:
                    _pop1()
                if late is not None:
                    for step in late:
                        step()
                return outproj_ops(qb, zpair)

            def outproj_split(qb, zpair):
                """qb=3 variant: p0+p1 partials run early (PE bubbles during
                the last pair), only the short p2 pass waits on the final
                normalize."""
                qsl = slice(qb * QB, (qb + 1) * QB)
                partial = [None] * EC

                def pass1():
                    for e in range(EC):
                        st = {}
                        def mk(e, p, st=st):
                            def step():
                                if p == 0:
                                    st["ps"] = psA.tile(
                                        [128, QB], F32, tag="misc", bufs=2,
                                        name=f"opsa{qb}_{e}")
                                nc.tensor.matmul(
                                    st["ps"][:], R(wo[p][:, e * 128:(e + 1) * 128]),
                                    R(zpair[p][:]), start=(p == 0), stop=(p == 1))
                            return step
                        yield mk(e, 0)
                        yield mk(e, 1)
                        def fin(e, st=st):
                            def step():
                                t = op.tile([128, QB], F32, tag=f"partial{e}",
                                            name=f"partial{qb}_{e}")
                                partial[e] = t
                                nc.vector.tensor_copy(t[:], st["ps"][:])
                            return step
                        yield fin(e)

                def pass2():
                    for e in range(EC):
                        st = {}
                        def mk(e, st=st):
                            def step():
                                st["ps"] = psA.tile([128, QB], F32, tag="misc",
                                                    bufs=2, name=f"opsb{qb}_{e}")
                                nc.tensor.matmul(
                                    st["ps"][:], R(wo[2][:, e * 128:(e + 1) * 128]),
                                    R(zpair[2][:]), start=True, stop=True)
                            return step
                        yield mk(e)
                        def fin(e, st=st):
                            def step():
                                osb = op.tile([128, QB], DT, tag="osb",
                                              name=f"osb{qb}_{e}")
                                nc.vector.scalar_tensor_tensor(
                                    osb[:], st["ps"][:], bo[e],
                                    partial[e][:],
                                    op0=mybir.AluOpType.add,
                                    op1=mybir.AluOpType.add)
                                # tail: split the final-quarter stores across
                                # both HWDGE rings (issue + wire in parallel)
                                deng = nc.sync if e % 2 == 0 else nc.scalar
                                deng.dma_start(
                                    d_out[e * 128:(e + 1) * 128, qsl], osb[:])
                            return step
                        yield fin(e)
                return pass1, pass2

            def outproj_ops(qb, zpair):
                qsl = slice(qb * QB, (qb + 1) * QB)
                for e in range(EC):
                    st = {}
                    def mk(e, p):
                        def step():
                            if p == 0:
                                st["ps"] = psA.tile([128, QB], F32, tag="misc",
                                                    bufs=2, name=f"ops{qb}_{e}")
                            nc.tensor.matmul(
                                st["ps"][:], R(wo[p][:, e * 128:(e + 1) * 128]),
                                R(zpair[p][:]),
                                start=(p == 0), stop=(p == PAIRS - 1))
                        return step
                    for p in range(PAIRS):
                        yield mk(e, p)
                    def fin(e):
                        def step():
                            osb = op.tile([128, QB], DT, tag="osb",
                                          name=f"osb{qb}_{e}")
                            nc.vector.tensor_scalar_add(osb[:], st["ps"][:],
                                                        bo[e])
                            nc.sync.dma_start(d_out[e * 128:(e + 1) * 128, qsl],
                                              osb[:])
                        return step
                    yield fin(e)

            for step in kq_ops(0):
                step()
            for step in v_ops(0):
                step()
            carry = []
            for qb in range(NQB):
                if qb == 0:
                    drain = list(carry) + list(kq_ops(1))
                    oops = attention(qb, drain=iter(drain), late=v_ops(1))
                    carry = list(oops)
                elif qb == 1:
                    drain = list(carry) + list(kq_ops(2))
                    oops = attention(qb, drain=iter(drain), late=v_ops(2))
                    carry = list(oops)
                elif qb == 2:
                    # quarter balance: attention(2) is PE-drain-saturated while
                    # attention(3) is ACT-bound with idle PE — keep only the
                    # next quarter's pair-0 K/Q chains here and push pairs 1-2
                    # plus the V chains into attention(3)'s bubbles (gated by
                    # markers so chains are emitted before their consumers)
                    drain = list(carry) + list(kq_pair_ops(3, 0))
                    oops = attention(qb, drain=iter(drain))
                    carry = list(oops)
                else:
                    zpair_last = [zp.tile([128, QB], DT, tag=f"zp{p}",
                                          name=f"zpL{p}") for p in range(PAIRS)]
                    pass1, pass2 = outproj_split(qb, zpair_last)
                    drain = (list(v_ops(qb)) + [("m", "vt3")]
                             + list(kq_pair_ops(qb, 1)) + [("m", "kq31")]
                             + list(kq_pair_ops(qb, 2)) + [("m", "kq32")]
                             + list(carry))
                    attention(qb, drain=iter(drain),
                              last_pair_drain=pass1(),
                              zpair_override=zpair_last,
                              prereq={1: "kq31", 2: "kq32"},
                              vt_gate="vt3")
                    for step in pass2():
                        step()

    nc.compile()
    return nc


def _get_nc():
    if _g["nc"] is None:
        _g["nc"] = _build()
    return _g["nc"]


def _make_in_maps(inputs):
    x = np.asarray(inputs["normalized_resid_pre"], dtype=np.float32)
    W_Q = np.asarray(inputs["W_Q"], dtype=np.float32)
    W_K = np.asarray(inputs["W_K"], dtype=np.float32)
    W_V = np.asarray(inputs["W_V"], dtype=np.float32)
    W_O = np.asarray(inputs["W_O"], dtype=np.float32)
    b_Q = np.asarray(inputs["b_Q"], dtype=np.float32)
    b_K = np.asarray(inputs["b_K"], dtype=np.float32)
    b_V = np.asarray(inputs["b_V"], dtype=np.float32)
    b_O = np.asarray(inputs["b_O"], dtype=np.float32)
    dt = _np_dt()

    # 0/1 keep-masks for the diagonal 128-col chunk of each band offset,
    # [4*128, 2*128]; both 128-col halves carry the same triangular pattern
    # (they hold the two heads of a pair). Same for all 4 offsets, but kept
    # per-offset so each band block multiplies its own tile.
    # [128, 4 offsets x 2 heads x 128]: same triangular pattern per offset
    mask = np.zeros((KB, 4 * 2 * KB), dtype=dt)
    for o in range(4):
        for dk in range(KB):
            for half in range(2):
                base = o * 2 * KB + half * KB
                mask[dk, base + dk: base + KB] = 1.0
    in_maps = []
    for c in range(8):
        b = c // 2
        hs = (c % 2) * HPC
        heads = list(range(hs, hs + HPC))
        def pack(w):
            # [E, C] -> [128, EC*C] with column block e holding rows e*128..
            C = w.shape[1]
            return np.ascontiguousarray(
                w.reshape(EC, 128, C).transpose(1, 0, 2).reshape(128, EC * C))

        wq = np.concatenate(
            [pack(np.concatenate([W_Q[heads[2 * p]], W_Q[heads[2 * p + 1]]], axis=1))
             for p in range(PAIRS)], axis=0)             # [3*128, 768]
        wk = np.concatenate(
            [pack(np.concatenate([W_K[heads[2 * p]], W_K[heads[2 * p + 1]]], axis=1))
             for p in range(PAIRS)], axis=0)
        wv = np.zeros((E, VW * HPC), dtype=np.float32)
        bv = np.zeros((128, VW * HPC), dtype=np.float32)
        for h in range(HPC):
            wv[:, h * VW: h * VW + H] = W_V[heads[h]]
            bv[:, h * VW: h * VW + H] = b_V[heads[h]][None, :]
            bv[:, h * VW + H] = 1.0
        wv = pack(wv)                                    # [128, 6*390]
        wo = np.concatenate(
            [np.concatenate([W_O[heads[2 * p]], W_O[heads[2 * p + 1]]], axis=0)
             for p in range(PAIRS)], axis=0)             # [3*128, 768]
        # bqk [128, 6]: cols 0..2 = bq per pair, cols 3..5 = bk per pair
        bqk = np.zeros((128, 2 * PAIRS), dtype=np.float32)
        for p in range(PAIRS):
            bqk[:, p] = np.concatenate(
                [b_Q[heads[2 * p]], b_Q[heads[2 * p + 1]]])
            bqk[:, PAIRS + p] = np.concatenate(
                [b_K[heads[2 * p]], b_K[heads[2 * p + 1]]])
        # bo [128, 6]: col e = rows e*128..(e+1)*128 of b_O/2
        bo2 = np.ascontiguousarray(
            (b_O / 2.0).reshape(EC, 128).T)
        in_maps.append({
            "xT": np.ascontiguousarray(x[b].T).astype(dt),
            "wq": wq.astype(dt), "wk": wk.astype(dt),
            "wv": wv.astype(dt), "wo": wo.astype(dt),
            "bqk": bqk, "bv": bv, "bo": bo2,
            "mask": mask, "iden": np.eye(128, dtype=dt),
        })
    return in_maps


def _gather(results):
    out = np.empty((B, S, E), dtype=np.float32)
    for b in range(B):
        acc = results[2 * b]["outT"].astype(np.float32) + \
              results[2 * b + 1]["outT"].astype(np.float32)
        out[b] = acc.T
    return out


def run(inputs, trace=False):
    """Returns (output, BassKernelResults)."""
    from concourse.bass_utils import run_bass_kernel_spmd

    if trace:
        _install_ntff_shim()
    nc = _get_nc()
    in_maps = _make_in_maps(inputs)
    res = run_bass_kernel_spmd(nc, in_maps, core_ids=list(range(8)), trace=trace)
    return _gather(res.results), res


def kernel(**inputs):
    out, _ = run(inputs, trace=False)
    return out


def _install_ntff_shim():
    """The agent image's antenv lacks axon_hooks; recreate it so
    run_bass_kernel_spmd(trace=True) can capture NTFF profiles."""
    import types, ctypes, contextlib

    if "antenv.axon_hooks" in sys.modules:
        return
    so_path = "/opt/axon/libaxon_pjrt.so"
    try:
        lib = ctypes.CDLL(so_path)
        lib.axon_start_nrt_profile.argtypes = [ctypes.POINTER(ctypes.c_int64),
                                              ctypes.c_size_t]
        lib.axon_start_nrt_profile.restype = ctypes.c_int64
        lib.axon_stop_nrt_profile.argtypes = [ctypes.c_char_p]
        lib.axon_stop_nrt_profile.restype = ctypes.c_int64
    except (OSError, AttributeError):
        return

    @contextlib.contextmanager
    def _hook(output_dir, device_ids):
        import jax

        jax.devices()
        if device_ids:
            ids = (ctypes.c_int64 * len(device_ids))(*device_ids)
            rc = lib.axon_start_nrt_profile(ids, len(device_ids))
        else:
            rc = lib.axon_start_nrt_profile(None, 0)
        if rc != 0:
            raise RuntimeError(f"axon_start_nrt_profile rc={rc}")
        try:
            yield
        finally:
            n = lib.axon_stop_nrt_profile(str(output_dir).encode())
            print(f"ntff profile: {n} file(s) -> {output_dir}", file=sys.stderr)

    mod = types.ModuleType("antenv.axon_hooks")
    mod.get_axon_ntff_profile_hook = lambda: _hook
    sys.modules["antenv.axon_hooks"] = mod
    # avoid S3 upload attempts from the trace post-processing
    from concourse import bass_utils as bu

    bu.upload_artifacts = lambda tmpdir: f"local:{tmpdir}"



# revision 97
# speedup vs baseline: 1.0016x; 1.0016x over previous
"""Causal multi-head attention (B=4, S=2048, E=768, N=12 heads, H=64) on 8
Trainium2 NeuronCores.

Sharding: core c handles batch c//2 and heads (c%2)*6 .. +6 (tensor parallel
over heads within a batch pair). No collectives: each core emits a partial
out^T = (sum over its 6 heads of z @ W_O) + b_O/2, and the host sums the two
partials per batch and transposes back.

Layout: all device math runs in a transposed layout (seq on the free axis):
  xT [E, S] per batch (host-transposed)
  Q^T/K^T per head-pair  [128 (2x64h), S] in per-512-column tiles
  V natural [S, 65*6]  (65th column per head is all-ones -> PV matmul row 64
                        accumulates the softmax denominator for free)
  S^T [k, q] scores, both heads of a pair computed concurrently in the PE
  array via tile_position row groups; P = exp(scale*S^T), diagonal blocks
  multiplied by a 0/1 keep-mask; z^T [64, q] normalized by 1/denominator
  (fast DVE reciprocal + gpsimd partition_broadcast);
  out^T [E, S] accumulated over head pairs (K=128 contraction).

Scheduling: projection blocks for query block qb+1 and the output projection
for qb are emitted as single-instruction closures drained into attention(qb+1)
iterations, filling PE bubbles left by the ACT-bound exp pipeline.

Perf notes (this revision, ~200us vs 230us predecessor):
- HAM warm-up: ~24 junk matmuls into the idle 'z' PSUM banks flip the PE
  clock gate from 1.2 to 2.4 GHz before the first DMA-gated real matmul.
- Causal band trim: diagonal-band blocks (kb >= 4qb) compute only q-columns
  >= (kb-4qb)*128 in QK / exp (strided [128,2,N] AP) / PV; the mask multiply
  shrinks to the [128, 2x128] diagonal chunk.
- Engine rebalance: K/Q bias copies and out-proj bias adds moved ACT -> DVE;
  softmax reciprocal+broadcast run immediately after each pair's PV (only the
  normalize multiply is deferred); final-quarter stores split across both
  HWDGE rings.
- Quarter balance: attention(0..2) are PE-drain-saturated while attention(3)
  is ACT-bound with idle PE, so quarter 3's K/Q pair-1/2 and V chains drain
  into attention(3) itself, gated by emission-order markers (prereq per pair,
  vt_gate before the first band PV). Pushing more than this into (1)/(2)
  measured worse — they have no spare bubble capacity.
- xT arrives in 8 batched [128, 3*512] DMAs (descriptor issue on the sync
  engine costs ~600ns each; 24 issues serialized the early phase); wv and the
  late-needed wo/bo ride the scalar ring (wo appended AFTER the critical
  wk/wq — interleaving them measured +40us), bv/masks the sync ring right
  after xT quarter 0; gpsimd touches no SWDGE queue at all (it moves only
  ~80GB/s, starved the first V chains, and its exit dge_drain shrinks when
  unused); tiny bias/mask tensors are packed into single descriptors;
  outputs store as bf16 (halves 6MB of store traffic; rel err 4.1e-3 ->
  4.4e-3, gate 2e-2).
Pitfalls learned on HW: PSUM reads need 32-aligned partition bases; custom
DVE ops and gpsimd partition_broadcast cannot partition-shift; DMA cannot
read PSUM; a gpsimd tensor op amid SWDGE traffic forces a ~16us dge_drain.
"""

import sys

sys.path.insert(0, "/opt/trn_rl_repo")

import numpy as np

B, S, E = 4, 2048, 768
N_HEADS, H = 12, 64
HPC = 6           # heads per core
PAIRS = 3         # head pairs per core
EC = E // 128     # 6 e-chunks
QB = 512          # query block (free dim of most matmuls)
NQB = S // QB     # 4
KB = 128          # key sub-block (partition dim of S^T)
SC = S // 128     # 16 s-chunks for V
VW = 65           # V width per head incl. ones column
SCALE = 1.0 / np.sqrt(np.float32(H))

# Compute dtype for PE-facing tensors: "float32r" streams fp32 at full PE rate
# when the moving dim >= 256; "float32" is exact but 4 cycles/row; "bfloat16"
# halves SBUF footprint.
COMPUTE_DT = "bfloat16"

_g = {"nc": None}


def _np_dt():
    if COMPUTE_DT == "bfloat16":
        import ml_dtypes

        return ml_dtypes.bfloat16
    return np.float32


def _build():
    from concourse import bacc, tile, mybir

    F32 = mybir.dt.float32
    DT = getattr(mybir.dt, COMPUTE_DT)
    def R(ap):
        return ap

    nc = bacc.Bacc("TRN2", target_bir_lowering=False, debug=False, num_devices=8)

    d_xT = nc.dram_tensor("xT", [E, S], DT, kind="ExternalInput").ap()
    d_wq = nc.dram_tensor("wq", [PAIRS * 128, E], DT, kind="ExternalInput").ap()
    d_wk = nc.dram_tensor("wk", [PAIRS * 128, E], DT, kind="ExternalInput").ap()
    d_wv = nc.dram_tensor("wv", [128, VW * HPC * EC], DT, kind="ExternalInput").ap()
    d_wo = nc.dram_tensor("wo", [PAIRS * 128, E], DT, kind="ExternalInput").ap()
    # bq|bk packed as columns; bo chunks as columns: tiny per-tensor DMA
    # descriptors cost ~600ns of engine issue time each
    d_bqk = nc.dram_tensor("bqk", [128, 2 * PAIRS], F32, kind="ExternalInput").ap()
    d_bv = nc.dram_tensor("bv", [128, VW * HPC], F32, kind="ExternalInput").ap()
    d_bo = nc.dram_tensor("bo", [128, EC], F32, kind="ExternalInput").ap()
    d_mask = nc.dram_tensor("mask", [KB, 4 * 2 * KB], DT, kind="ExternalInput").ap()
    d_iden = nc.dram_tensor("iden", [128, 128], DT, kind="ExternalInput").ap()
    d_out = nc.dram_tensor("outT", [E, S], DT, kind="ExternalOutput").ap()

    Exp = mybir.ActivationFunctionType.Exp
    Copy = mybir.ActivationFunctionType.Copy

    with tile.TileContext(nc) as tc:
        with tc.tile_pool(name="persist", bufs=1) as pp, \
             tc.tile_pool(name="work", bufs=4) as wp, \
             tc.tile_pool(name="zsb", bufs=3) as zp, \
             tc.tile_pool(name="outsb", bufs=4) as op, \
             tc.tile_pool(name="psA", bufs=1, space="PSUM") as psA:

            # ---- HAM warm-up --------------------------------------------------
            # The PE clock gate (HAM) starts at K=4/8 (1.2 GHz) and only
            # promotes to 8/8 after ~3.4us of sustained PE activity. Real work
            # can't start until the first DMAs land (~10.5us: ~6us engine
            # preamble + DGE wake + transfer), so without a primer the whole
            # first attention block runs at half clock. Issue junk matmuls on
            # a memset tile to flip the HAM before real work arrives.
            warm = pp.tile([128, QB], DT, tag="warm", name="warm")
            nc.gpsimd.memset(warm[:], 0.0)
            # junk targets the 'z' psum banks: those are first needed by
            # attention(0)'s PV (~18us), so the warm-up never blocks the
            # first projection chains (which use the 'misc' banks)
            wps = [psA.tile([VW, QB], F32, tag="z", bufs=2, name=f"warmps{i}")
                   for i in range(2)]
            for i in range(24):
                nc.tensor.matmul(wps[i % 2][:], warm[:, 0:VW], warm[:],
                                 start=True, stop=True)

            # ---- static tiles -------------------------------------------------
            # DMA routing: weights for the first projections go on the ACT
            # HWDGE ring, xT halves on the SP ring (the two rings run in
            # parallel), and everything not needed until later (W_O, b_O,
            # masks, V weights/biases) on the gpsimd SWDGE queues.
            wq, wk, wo = [], [], []
            for p in range(PAIRS):
                tk = pp.tile([128, E], DT, tag=f"wk{p}", name=f"wk{p}")
                nc.scalar.dma_start(tk[:], d_wk[p * 128:(p + 1) * 128, :])
                wk.append(tk)
                tq = pp.tile([128, E], DT, tag=f"wq{p}", name=f"wq{p}")
                nc.scalar.dma_start(tq[:], d_wq[p * 128:(p + 1) * 128, :])
                wq.append(tq)
                to = pp.tile([128, E], DT, tag=f"wo{p}", name=f"wo{p}")
                wo.append(to)
            bqk = pp.tile([128, 2 * PAIRS], F32, tag="bqk", name="bqk")
            nc.scalar.dma_start(bqk[:], d_bqk[:, :])
            bq = [bqk[:, p:p + 1] for p in range(PAIRS)]
            bk = [bqk[:, PAIRS + p:PAIRS + p + 1] for p in range(PAIRS)]
            bo = []
            wv_all = pp.tile([128, VW * HPC * EC], DT, tag="wv", name="wv_all")
            nc.scalar.dma_start(wv_all[:], d_wv[:, :])
            wv = [wv_all[:, e * VW * HPC:(e + 1) * VW * HPC] for e in range(EC)]
            # xT in 8 batched transfers ([128, 3, 512] each): one dma_start
            # costs ~600ns of engine time, so 8 descriptors instead of 24
            # frees the sync engine ~10us earlier; half-quarter granularity
            # still lets the first projection chain start on e0-2.
            # sync-ring order matters (transfers run in order at wire speed):
            # xT quarter 0 first (first projections), then wv/bv/masks
            # (needed ~18-25us; the gpsimd SWDGE queue is ~80GB/s-slow and
            # the scalar ring's issue flow-control would hold them to ~18us),
            # then xT quarters 1-3 (needed at ~19/~40/~60us).
            d_xT3 = d_xT.rearrange("(e p) s -> p e s", p=128)
            xbig = [[None, None] for _ in range(4)]
            for quarter in range(4):
                for half in range(2):
                    xbig[quarter][half] = pp.tile(
                        [128, 3 * QB], DT, tag=f"xt{quarter}_{half}",
                        name=f"xt{quarter}_{half}")

            def _xtq_dma(quarter):
                hs = slice(quarter * QB, (quarter + 1) * QB)
                for half in range(2):
                    t = xbig[quarter][half]
                    nc.sync.dma_start(
                        t[:].rearrange("p (e s) -> p e s", s=QB),
                        d_xT3[:, 3 * half:3 * half + 3, hs])



            def xt(e, sb, c0=0, c1=QB):
                t = xbig[sb][e // 3]
                base = (e % 3) * QB
                return t[:, base + c0:base + c1]
            for p in range(PAIRS):
                nc.scalar.dma_start(wo[p][:], d_wo[p * 128:(p + 1) * 128, :])
            boall = pp.tile([128, EC], F32, tag="bo", name="boall")
            nc.scalar.dma_start(boall[:], d_bo[:, :])
            iden = pp.tile([128, 128], DT, tag="iden", name="iden")
            nc.scalar.dma_start(iden[:], d_iden[:, :])
            bo = [boall[:, e:e + 1] for e in range(EC)]
            bv = pp.tile([128, VW * HPC], F32, tag="bv")
            maskall = pp.tile([KB, 4 * 2 * KB], DT, tag="mask", name="maskall")
            masks = [maskall[:, o * 2 * KB:(o + 1) * 2 * KB] for o in range(4)]

            _xtq_dma(0)
            nc.sync.dma_start(bv[:], d_bv[:, :])
            nc.sync.dma_start(maskall[:], d_mask[:, :])
            for quarter in range(1, 4):
                _xtq_dma(quarter)

            kt = [[pp.tile([128, QB], DT, tag=f"kt{p}_{sb}", name=f"kt{p}_{sb}")
                   for sb in range(NQB)] for p in range(PAIRS)]
            qt = [[pp.tile([128, QB], DT, tag=f"qt{p}_{sb}", name=f"qt{p}_{sb}")
                   for sb in range(NQB)] for p in range(PAIRS)]
            vt = [pp.tile([128, VW * HPC], DT, tag=f"vt{s}", name=f"vt{s}") for s in range(SC)]

            Iden = mybir.ActivationFunctionType.Identity

            def _mk_chain():
                def chain(name, width, lhs_of_e, rhs_of_e, copy_out):
                    st = {}
                    def mk(e):
                        def step():
                            if e == 0:
                                st["ps"] = psA.tile(
                                    [128, width], F32, tag="misc", bufs=2,
                                    name=name)
                            nc.tensor.matmul(st["ps"][:],
                                             R(lhs_of_e(e)), R(rhs_of_e(e)),
                                             start=(e == 0), stop=(e == EC - 1))
                        return step
                    for e in range(EC):
                        yield mk(e)
                    yield lambda: copy_out(st["ps"])
                return chain

            def kq_pair_ops(sb, p, chain=None):
                # bias-add copies on DVE, keeping ACT free for the exp pipeline
                chain = chain or _mk_chain()
                kcopy = lambda ps, p=p, sb=sb: nc.vector.tensor_scalar_add(
                    kt[p][sb][:], ps[:], bk[p])
                qcopy = lambda ps, p=p, sb=sb: nc.vector.tensor_scalar_add(
                    qt[p][sb][:], ps[:], bq[p])
                yield from chain(
                    f"kps{p}_{sb}", QB,
                    lambda e, p=p: wk[p][:, e * 128:(e + 1) * 128],
                    lambda e, sb=sb: xt(e, sb), kcopy)
                yield from chain(
                    f"qps{p}_{sb}", QB,
                    lambda e, p=p: wq[p][:, e * 128:(e + 1) * 128],
                    lambda e, sb=sb: xt(e, sb), qcopy)

            def kq_ops(sb, chain=None):
                for p in range(PAIRS):
                    yield from kq_pair_ops(sb, p, chain)

            def v_ops(sb, chain=None):
                chain = chain or _mk_chain()
                for s in range(4 * sb, 4 * sb + 4):
                    yield from chain(
                        f"vps{s}", VW * HPC,
                        lambda e, sb=sb, s=s: xt(e, sb, (s % 4) * 128, (s % 4 + 1) * 128),
                        lambda e: wv[e],
                        lambda ps, s=s: nc.vector.tensor_add(
                            vt[s][:], ps[:], bv[:]))

            def make_normalize(qb, zpair):
                def normalize(head, zsb, bcast, unused=False):
                    # deferred: z * (1/denom), recip+broadcast already done.
                    # NB must stay on DVE: a gpsimd tensor op forces a ~16us
                    # dge_drain (SWDGE<->compute mode switch) on that engine.
                    p, sub = head // 2, head % 2
                    hsl = slice(sub * 64, sub * 64 + 64)
                    nc.vector.tensor_mul(zpair[p][hsl, :], zsb[:], bcast[:])
                return normalize

            def attention(qb, drain=None, late=None, last_pair_drain=None,
                          zpair_override=None, prereq=None, vt_gate=None):
                q0 = qb * QB
                qsl = slice(q0, q0 + QB)
                nkb = 4 * qb + 4
                # drain elements: zero-arg closures, or ("m", key) markers
                dq = list(drain) if drain is not None else []
                seen = set()
                iters = [PAIRS * max(nkb - 1, 1), 0]

                def _pop1():
                    el = dq.pop(0)
                    if isinstance(el, tuple):
                        seen.add(el[1])
                    else:
                        el()

                def drain_some():
                    if not dq:
                        return
                    n = max(1, -(-len(dq) // max(iters[0] - iters[1], 1)))
                    for _ in range(n):
                        if dq:
                            _pop1()
                    iters[1] += 1

                def drain_until(key):
                    # force-drain so a prerequisite chain is fully EMITTED
                    # before instructions that depend on it (emission order on
                    # an engine is execution order — a dep on a later
                    # instruction would deadlock)
                    while key not in seen and dq:
                        _pop1()
                zpair = zpair_override or [
                    zp.tile([128, QB], DT, tag=f"zp{p}", name=f"zp{p}_{qb}")
                    for p in range(PAIRS)]
                normalize = make_normalize(qb, zpair)
                pending = []
                for p in range(PAIRS):
                    if prereq and p in prereq:
                        drain_until(prereq[p])
                    zab = [psA.tile([VW, QB], F32, tag="z", bufs=2,
                                    name=f"zps{qb}_{2 * p + s}") for s in range(2)]

                    def qk(kb):
                        # both heads of the pair, concurrent via PE row groups.
                        # Diagonal-band blocks (kb >= 4qb) only need queries
                        # q >= (kb-4qb)*128: trim the streamed q range.
                        co = max(0, (kb - 4 * qb) * KB)
                        sps = psA.tile([KB, 2 * QB], F32, tag="s", bufs=2,
                                       name=f"sps{qb}_{p}_{kb}")
                        ktt = kt[p][kb // 4]
                        ksl = slice((kb % 4) * KB, (kb % 4 + 1) * KB)
                        nc.tensor.matmul(
                            sps[:, co:QB], R(ktt[0:64, ksl]), R(qt[p][qb][0:64, co:QB]),
                            start=True, stop=True, tile_position=(0, 0))
                        nc.tensor.matmul(
                            sps[:, QB + co:2 * QB], R(ktt[64:128, ksl]), R(qt[p][qb][64:128, co:QB]),
                            start=True, stop=True, tile_position=(64, 0))
                        return sps

                    def pv(kb, sps):
                        co = max(0, (kb - 4 * qb) * KB)
                        pt = wp.tile([KB, 2 * QB], DT, tag="p", bufs=6,
                                     name=f"pt{qb}_{p}_{kb}")
                        if co:
                            # strided [128, 2, QB-co] view covering both heads
                            sv = sps[:].rearrange("k (two q) -> k two q", two=2)[:, :, co:]
                            ptv = pt[:].rearrange("k (two q) -> k two q", two=2)[:, :, co:]
                            nc.scalar.activation(ptv, sv, Exp, scale=float(SCALE))
                        else:
                            nc.scalar.activation(pt[:], sps[:], Exp, scale=float(SCALE))
                        if kb >= 4 * qb:  # diagonal 128-col chunk: zero out k > q
                            o = kb - 4 * qb
                            ptd = pt[:].rearrange("k (two q) -> k two q", two=2)[:, :, co:co + KB]
                            mv = masks[o].rearrange("k (two q) -> k two q", two=2)
                            nc.vector.tensor_mul(ptd, ptd, mv)
                        for s in range(2):
                            nc.tensor.matmul(
                                zab[s][:, co:QB], R(vt[kb][:, (2 * p + s) * VW:(2 * p + s + 1) * VW]),
                                R(pt[:, s * QB + co:(s + 1) * QB]),
                                start=(kb == 0), stop=(kb == nkb - 1))

                    prev = qk(0)
                    for kb in range(1, nkb):
                        cur = qk(kb)
                        if vt_gate and kb - 1 == 4 * qb:
                            # band PV needs this quarter's vt chains emitted
                            drain_until(vt_gate)
                        pv(kb - 1, prev)
                        drain_some()
                        prev = cur
                        if kb == 2:
                            for args in pending:
                                normalize(*args)
                            pending = []
                            if p == PAIRS - 1 and last_pair_drain is not None:
                                dq.extend(last_pair_drain)
                    pv(nkb - 1, prev)
                    drain_some()

                    last = (qb == NQB - 1 and p == PAIRS - 1)
                    if last:
                        # tail: spread the readout chain across engines so the
                        # recip -> bcast -> mul critical path starts as early
                        # as possible (pass2 waits on the muls)
                        for s in range(2):
                            head = 2 * p + s
                            den = wp.tile([1, QB], F32, tag="den",
                                          name=f"den{qb}_{head}")
                            nc.scalar.activation(den[:], zab[s][64:65, :], Iden)
                            recipf = wp.tile([1, QB], F32, tag="recipf",
                                             name=f"recipf{qb}_{head}")
                            nc.vector.reciprocal_approx_fast(recipf[:], den[:])
                            zsb = wp.tile([64, QB], F32, tag="zc",
                                          name=f"zsb{qb}_{head}")
                            if s == 0:
                                nc.scalar.activation(zsb[:], zab[s][0:64, :], Iden)
                            else:
                                nc.vector.tensor_copy(zsb[:], zab[s][0:64, :])
                            bcast = wp.tile([64, QB], F32, tag="bcast",
                                            name=f"bcast{qb}_{head}")
                            nc.gpsimd.partition_broadcast(bcast[:], recipf[:])
                            pending.append((head, zsb, bcast, s == 0))
                    else:
                        # PSUM-freeing copies first (the next pair's PV waits
                        # on the zab banks), recip/broadcast after; in the
                        # PE-bound early quarters head 1's copies go to the
                        # then-idle ACT so the banks free ~2x sooner
                        zts = []
                        for s in range(2):
                            head = 2 * p + s
                            den = wp.tile([1, QB], F32, tag="den",
                                          name=f"den{qb}_{head}")
                            zsb = wp.tile([64, QB], F32, tag="zc",
                                          name=f"zsb{qb}_{head}")
                            if qb <= 1 and s == 1:
                                nc.scalar.activation(den[:], zab[s][64:65, :], Iden)
                                nc.scalar.activation(zsb[:], zab[s][0:64, :], Iden)
                            else:
                                nc.vector.tensor_copy(den[:], zab[s][64:65, :])
                                nc.vector.tensor_copy(zsb[:], zab[s][0:64, :])
                            zts.append((head, den, zsb))
                        for head, den, zsb in zts:
                            recipf = wp.tile([1, QB], F32, tag="recipf",
                                             name=f"recipf{qb}_{head}")
                            nc.vector.reciprocal_approx_fast(recipf[:], den[:])
                            bcast = wp.tile([64, QB], F32, tag="bcast",
                                            name=f"bcast{qb}_{head}")
                            nc.gpsimd.partition_broadcast(bcast[:], recipf[:])
                            pending.append((head, zsb, bcast, False))
                for args in pending:
                    normalize(*args)
                while dq:
                    _pop1()
                if late is not None:
                    for step in late:
                        step()
                return outproj_ops(qb, zpair)

            def outproj_split(qb, zpair):
                """qb=3 variant: p0+p1 partials run early (PE bubbles during
                the last pair), only the short p2 pass waits on the final
                normalize."""
                qsl = slice(qb * QB, (qb + 1) * QB)
                partial = [None] * EC

                def pass1():
                    for e in range(EC):
                        st = {}
                        def mk(e, p, st=st):
                            def step():
                                if p == 0:
                                    st["ps"] = psA.tile(
                                        [128, QB], F32, tag="misc", bufs=2,
                                        name=f"opsa{qb}_{e}")
                                nc.tensor.matmul(
                                    st["ps"][:], R(wo[p][:, e * 128:(e + 1) * 128]),
                                    R(zpair[p][:]), start=(p == 0), stop=(p == 1))
                            return step
                        yield mk(e, 0)
                        yield mk(e, 1)
                        def fin(e, st=st):
                            def step():
                                t = op.tile([128, QB], F32, tag=f"partial{e}",
                                            name=f"partial{qb}_{e}")
                                partial[e] = t
                                nc.vector.tensor_copy(t[:], st["ps"][:])
                            return step
                        yield fin(e)

                def pass2():
                    for e in range(EC):
                        st = {}
                        def mk(e, st=st):
                            def step():
                                st["ps"] = psA.tile([128, QB], F32, tag="misc",
                                                    bufs=2, name=f"opsb{qb}_{e}")
                                nc.tensor.matmul(
                                    st["ps"][:], R(wo[2][:, e * 128:(e + 1) * 128]),
                                    R(zpair[2][:]), start=True, stop=True)
                            return step
                        yield mk(e)
                        def fin(e, st=st):
                            def step():
                                osb = op.tile([128, QB], DT, tag="osb",
                                              name=f"osb{qb}_{e}")
                                nc.vector.scalar_tensor_tensor(
                                    osb[:], st["ps"][:], bo[e],
                                    partial[e][:],
                                    op0=mybir.AluOpType.add,
                                    op1=mybir.AluOpType.add)
                                # tail: split the final-quarter stores across
                                # both HWDGE rings (issue + wire in parallel)
                                deng = nc.sync if e % 2 == 0 else nc.scalar
                                deng.dma_start(
                                    d_out[e * 128:(e + 1) * 128, qsl], osb[:])
                            return step
                        yield fin(e)
                return pass1, pass2

            def outproj_ops(qb, zpair):
                qsl = slice(qb * QB, (qb + 1) * QB)
                for e in range(EC):
                    st = {}
                    def mk(e, p):
                        def step():
                            if p == 0:
                                st["ps"] = psA.tile([128, QB], F32, tag="misc",
                                                    bufs=2, name=f"ops{qb}_{e}")
                            nc.tensor.matmul(
                                st["ps"][:], R(wo[p][:, e * 128:(e + 1) * 128]),
                                R(zpair[p][:]),
                                start=(p == 0), stop=(p == PAIRS - 1))
                        return step
                    for p in range(PAIRS):
                        yield mk(e, p)
                    def fin(e):
                        def step():
                            osb = op.tile([128, QB], DT, tag="osb",
                                          name=f"osb{qb}_{e}")
                            nc.vector.tensor_scalar_add(osb[:], st["ps"][:],
                                                        bo[e])
                            nc.sync.dma_start(d_out[e * 128:(e + 1) * 128, qsl],
                                              osb[:])
                        return step
                    yield fin(e)

            for step in kq_ops(0):
                step()
            for step in v_ops(0):
                step()
            carry = []
            for qb in range(NQB):
                if qb == 0:
                    drain = list(carry) + list(kq_ops(1)) + list(v_ops(1))
                    oops = attention(qb, drain=iter(drain))
                    carry = list(oops)
                elif qb == 1:
                    drain = list(carry) + list(kq_ops(2)) + list(v_ops(2))
                    oops = attention(qb, drain=iter(drain))
                    carry = list(oops)
                elif qb == 2:
                    # quarter balance: attention(2) is PE-drain-saturated while
                    # attention(3) is ACT-bound with idle PE — keep only the
                    # next quarter's pair-0 K/Q chains here and push pairs 1-2
                    # plus the V chains into attention(3)'s bubbles (gated by
                    # markers so chains are emitted before their consumers)
                    drain = list(carry) + list(kq_pair_ops(3, 0))
                    oops = attention(qb, drain=iter(drain))
                    carry = list(oops)
                else:
                    zpair_last = [zp.tile([128, QB], DT, tag=f"zp{p}",
                                          name=f"zpL{p}") for p in range(PAIRS)]
                    pass1, pass2 = outproj_split(qb, zpair_last)
                    drain = (list(v_ops(qb)) + [("m", "vt3")]
                             + list(kq_pair_ops(qb, 1)) + [("m", "kq31")]
                             + list(kq_pair_ops(qb, 2)) + [("m", "kq32")]
                             + list(carry))
                    attention(qb, drain=iter(drain),
                              last_pair_drain=pass1(),
                              zpair_override=zpair_last,
                              prereq={1: "kq31", 2: "kq32"},
                              vt_gate="vt3")
                    for step in pass2():
                        step()

    nc.compile()
    return nc


def _get_nc():
    if _g["nc"] is None:
        _g["nc"] = _build()
    return _g["nc"]


def _make_in_maps(inputs):
    x = np.asarray(inputs["normalized_resid_pre"], dtype=np.float32)
    W_Q = np.asarray(inputs["W_Q"], dtype=np.float32)
    W_K = np.asarray(inputs["W_K"], dtype=np.float32)
    W_V = np.asarray(inputs["W_V"], dtype=np.float32)
    W_O = np.asarray(inputs["W_O"], dtype=np.float32)
    b_Q = np.asarray(inputs["b_Q"], dtype=np.float32)
    b_K = np.asarray(inputs["b_K"], dtype=np.float32)
    b_V = np.asarray(inputs["b_V"], dtype=np.float32)
    b_O = np.asarray(inputs["b_O"], dtype=np.float32)
    dt = _np_dt()

    # 0/1 keep-masks for the diagonal 128-col chunk of each band offset,
    # [4*128, 2*128]; both 128-col halves carry the same triangular pattern
    # (they hold the two heads of a pair). Same for all 4 offsets, but kept
    # per-offset so each band block multiplies its own tile.
    # [128, 4 offsets x 2 heads x 128]: same triangular pattern per offset
    mask = np.zeros((KB, 4 * 2 * KB), dtype=dt)
    for o in range(4):
        for dk in range(KB):
            for half in range(2):
                base = o * 2 * KB + half * KB
                mask[dk, base + dk: base + KB] = 1.0
    in_maps = []
    for c in range(8):
        b = c // 2
        hs = (c % 2) * HPC
        heads = list(range(hs, hs + HPC))
        def pack(w):
            # [E, C] -> [128, EC*C] with column block e holding rows e*128..
            C = w.shape[1]
            return np.ascontiguousarray(
                w.reshape(EC, 128, C).transpose(1, 0, 2).reshape(128, EC * C))

        wq = np.concatenate(
            [pack(np.concatenate([W_Q[heads[2 * p]], W_Q[heads[2 * p + 1]]], axis=1))
             for p in range(PAIRS)], axis=0)             # [3*128, 768]
        wk = np.concatenate(
            [pack(np.concatenate([W_K[heads[2 * p]], W_K[heads[2 * p + 1]]], axis=1))
             for p in range(PAIRS)], axis=0)
        wv = np.zeros((E, VW * HPC), dtype=np.float32)
        bv = np.zeros((128, VW * HPC), dtype=np.float32)
        for h in range(HPC):
            wv[:, h * VW: h * VW + H] = W_V[heads[h]]
            bv[:, h * VW: h * VW + H] = b_V[heads[h]][None, :]
            bv[:, h * VW + H] = 1.0
        wv = pack(wv)                                    # [128, 6*390]
        wo = np.concatenate(
            [np.concatenate([W_O[heads[2 * p]], W_O[heads[2 * p + 1]]], axis=0)
             for p in range(PAIRS)], axis=0)             # [3*128, 768]
        # bqk [128, 6]: cols 0..2 = bq per pair, cols 3..5 = bk per pair
        bqk = np.zeros((128, 2 * PAIRS), dtype=np.float32)
        for p in range(PAIRS):
            bqk[:, p] = np.concatenate(
                [b_Q[heads[2 * p]], b_Q[heads[2 * p + 1]]])
            bqk[:, PAIRS + p] = np.concatenate(
                [b_K[heads[2 * p]], b_K[heads[2 * p + 1]]])
        # bo [128, 6]: col e = rows e*128..(e+1)*128 of b_O/2
        bo2 = np.ascontiguousarray(
            (b_O / 2.0).reshape(EC, 128).T)
        in_maps.append({
            "xT": np.ascontiguousarray(x[b].T).astype(dt),
            "wq": wq.astype(dt), "wk": wk.astype(dt),
            "wv": wv.astype(dt), "wo": wo.astype(dt),
            "bqk": bqk, "bv": bv, "bo": bo2,
            "mask": mask, "iden": np.eye(128, dtype=dt),
        })
    return in_maps


def _gather(results):
    out = np.empty((B, S, E), dtype=np.float32)
    for b in range(B):
        acc = results[2 * b]["outT"].astype(np.float32) + \
              results[2 * b + 1]["outT"].astype(np.float32)
        out[b] = acc.T
    return out


def run(inputs, trace=False):
    """Returns (output, BassKernelResults)."""
    from concourse.bass_utils import run_bass_kernel_spmd

    if trace:
        _install_ntff_shim()
    nc = _get_nc()
    in_maps = _make_in_maps(inputs)
    res = run_bass_kernel_spmd(nc, in_maps, core_ids=list(range(8)), trace=trace)
    return _gather(res.results), res


def kernel(**inputs):
    out, _ = run(inputs, trace=False)
    return out


def _install_ntff_shim():
    """The agent image's antenv lacks axon_hooks; recreate it so
    run_bass_kernel_spmd(trace=True) can capture NTFF profiles."""
    import types, ctypes, contextlib

    if "antenv.axon_hooks" in sys.modules:
        return
    so_path = "/opt/axon/libaxon_pjrt.so"
    try:
        lib = ctypes.CDLL(so_path)
        lib.axon_start_nrt_profile.argtypes = [ctypes.POINTER(ctypes.c_int64),
                                              ctypes.c_size_t]
        lib.axon_start_nrt_profile.restype = ctypes.c_int64
        lib.axon_stop_nrt_profile.argtypes = [ctypes.c_char_p]
        lib.axon_stop_nrt_profile.restype = ctypes.c_int64
    except (OSError, AttributeError):
        return

    @contextlib.contextmanager
    def _hook(output_dir, device_ids):
        import jax

        jax.devices()
        if device_ids:
            ids = (ctypes.c_int64 * len(device_ids))(*device_ids)
            rc = lib.axon_start_nrt_profile(ids, len(device_ids))
        else:
            rc = lib.axon_start_nrt_profile(None, 0)
        if rc != 0:
            raise RuntimeError(f"axon_start_nrt_profile rc={rc}")
        try:
            yield
        finally:
            n = lib.axon_stop_nrt_profile(str(output_dir).encode())
            print(f"ntff profile: {n} file(s) -> {output_dir}", file=sys.stderr)

    mod = types.ModuleType("antenv.axon_hooks")
    mod.get_axon_ntff_profile_hook = lambda: _hook
    sys.modules["antenv.axon_hooks"] = mod
    # avoid S3 upload attempts from the trace post-processing
    from concourse import bass_utils as bu

    bu.upload_artifacts = lambda tmpdir: f"local:{tmpdir}"



# revision 98
# speedup vs baseline: 1.0197x; 1.0180x over previous
"""Causal multi-head attention (B=4, S=2048, E=768, N=12 heads, H=64) on 8
Trainium2 NeuronCores.

Sharding: core c handles batch c//2 and heads (c%2)*6 .. +6 (tensor parallel
over heads within a batch pair). No collectives: each core emits a partial
out^T = (sum over its 6 heads of z @ W_O) + b_O/2, and the host sums the two
partials per batch and transposes back.

Layout: all device math runs in a transposed layout (seq on the free axis):
  xT [E, S] per batch (host-transposed)
  Q^T/K^T per head-pair  [128 (2x64h), S] in per-512-column tiles
  V natural [S, 65*6]  (65th column per head is all-ones -> PV matmul row 64
                        accumulates the softmax denominator for free)
  S^T [k, q] scores, both heads of a pair computed concurrently in the PE
  array via tile_position row groups; P = exp(scale*S^T), diagonal blocks
  multiplied by a 0/1 keep-mask; z^T [64, q] normalized by 1/denominator
  (fast DVE reciprocal + gpsimd partition_broadcast);
  out^T [E, S] accumulated over head pairs (K=128 contraction).

Scheduling: projection blocks for query block qb+1 and the output projection
for qb are emitted as single-instruction closures drained into attention(qb+1)
iterations, filling PE bubbles left by the ACT-bound exp pipeline.

Perf notes (this revision, ~200us vs 230us predecessor):
- HAM warm-up: ~24 junk matmuls into the idle 'z' PSUM banks flip the PE
  clock gate from 1.2 to 2.4 GHz before the first DMA-gated real matmul.
- Causal band trim: diagonal-band blocks (kb >= 4qb) compute only q-columns
  >= (kb-4qb)*128 in QK / exp (strided [128,2,N] AP) / PV; the mask multiply
  shrinks to the [128, 2x128] diagonal chunk.
- Engine rebalance: K/Q bias copies and out-proj bias adds moved ACT -> DVE;
  softmax reciprocal+broadcast run immediately after each pair's PV (only the
  normalize multiply is deferred); final-quarter stores split across both
  HWDGE rings.
- Quarter balance: attention(0..2) are PE-drain-saturated while attention(3)
  is ACT-bound with idle PE, so quarter 3's K/Q pair-1/2 and V chains drain
  into attention(3) itself, gated by emission-order markers (prereq per pair,
  vt_gate before the first band PV). Pushing more than this into (1)/(2)
  measured worse — they have no spare bubble capacity.
- xT arrives in 8 batched [128, 3*512] DMAs (descriptor issue on the sync
  engine costs ~600ns each; 24 issues serialized the early phase); wv and the
  late-needed wo/bo ride the scalar ring (wo appended AFTER the critical
  wk/wq — interleaving them measured +40us), bv/masks the sync ring right
  after xT quarter 0; gpsimd touches no SWDGE queue at all (it moves only
  ~80GB/s, starved the first V chains, and its exit dge_drain shrinks when
  unused); tiny bias/mask tensors are packed into single descriptors;
  outputs store as bf16 (halves 6MB of store traffic; rel err 4.1e-3 ->
  4.4e-3, gate 2e-2).
Pitfalls learned on HW: PSUM reads need 32-aligned partition bases; custom
DVE ops and gpsimd partition_broadcast cannot partition-shift; DMA cannot
read PSUM; a gpsimd tensor op amid SWDGE traffic forces a ~16us dge_drain.
"""

import sys

sys.path.insert(0, "/opt/trn_rl_repo")

import numpy as np

B, S, E = 4, 2048, 768
N_HEADS, H = 12, 64
HPC = 6           # heads per core
PAIRS = 3         # head pairs per core
EC = E // 128     # 6 e-chunks
QB = 512          # query block (free dim of most matmuls)
NQB = S // QB     # 4
KB = 128          # key sub-block (partition dim of S^T)
SC = S // 128     # 16 s-chunks for V
VW = 65           # V width per head incl. ones column
SCALE = 1.0 / np.sqrt(np.float32(H))

# Compute dtype for PE-facing tensors: "float32r" streams fp32 at full PE rate
# when the moving dim >= 256; "float32" is exact but 4 cycles/row; "bfloat16"
# halves SBUF footprint.
COMPUTE_DT = "bfloat16"

_g = {"nc": None}


def _np_dt():
    if COMPUTE_DT == "bfloat16":
        import ml_dtypes

        return ml_dtypes.bfloat16
    return np.float32


def _build():
    from concourse import bacc, tile, mybir

    F32 = mybir.dt.float32
    DT = getattr(mybir.dt, COMPUTE_DT)
    def R(ap):
        return ap

    nc = bacc.Bacc("TRN2", target_bir_lowering=False, debug=False, num_devices=8)

    d_xT = nc.dram_tensor("xT", [E, S], DT, kind="ExternalInput").ap()
    d_wq = nc.dram_tensor("wq", [PAIRS * 128, E], DT, kind="ExternalInput").ap()
    d_wk = nc.dram_tensor("wk", [PAIRS * 128, E], DT, kind="ExternalInput").ap()
    d_wv = nc.dram_tensor("wv", [128, VW * HPC * EC], DT, kind="ExternalInput").ap()
    d_wo = nc.dram_tensor("wo", [PAIRS * 128, E], DT, kind="ExternalInput").ap()
    # bq|bk packed as columns; bo chunks as columns: tiny per-tensor DMA
    # descriptors cost ~600ns of engine issue time each
    d_bqk = nc.dram_tensor("bqk", [128, 2 * PAIRS], F32, kind="ExternalInput").ap()
    d_bv = nc.dram_tensor("bv", [128, VW * HPC], F32, kind="ExternalInput").ap()
    d_bo = nc.dram_tensor("bo", [128, EC], F32, kind="ExternalInput").ap()
    d_mask = nc.dram_tensor("mask", [KB, 4 * 2 * KB], DT, kind="ExternalInput").ap()
    d_iden = nc.dram_tensor("iden", [128, 128], DT, kind="ExternalInput").ap()
    d_out = nc.dram_tensor("outT", [E, S], DT, kind="ExternalOutput").ap()

    Exp = mybir.ActivationFunctionType.Exp
    Copy = mybir.ActivationFunctionType.Copy

    with tile.TileContext(nc) as tc:
        with tc.tile_pool(name="persist", bufs=1) as pp, \
             tc.tile_pool(name="work", bufs=4) as wp, \
             tc.tile_pool(name="zsb", bufs=3) as zp, \
             tc.tile_pool(name="outsb", bufs=4) as op, \
             tc.tile_pool(name="psA", bufs=1, space="PSUM") as psA:

            # ---- HAM warm-up --------------------------------------------------
            # The PE clock gate (HAM) starts at K=4/8 (1.2 GHz) and only
            # promotes to 8/8 after ~3.4us of sustained PE activity. Real work
            # can't start until the first DMAs land (~10.5us: ~6us engine
            # preamble + DGE wake + transfer), so without a primer the whole
            # first attention block runs at half clock. Issue junk matmuls on
            # a memset tile to flip the HAM before real work arrives.
            warm = pp.tile([128, QB], DT, tag="warm", name="warm")
            nc.gpsimd.memset(warm[:], 0.0)
            # junk targets the 'z' psum banks: those are first needed by
            # attention(0)'s PV (~18us), so the warm-up never blocks the
            # first projection chains (which use the 'misc' banks)
            wps = [psA.tile([VW, QB], F32, tag="z", bufs=2, name=f"warmps{i}")
                   for i in range(2)]
            for i in range(24):
                nc.tensor.matmul(wps[i % 2][:], warm[:, 0:VW], warm[:],
                                 start=True, stop=True)

            # ---- static tiles -------------------------------------------------
            # DMA routing: weights for the first projections go on the ACT
            # HWDGE ring, xT halves on the SP ring (the two rings run in
            # parallel), and everything not needed until later (W_O, b_O,
            # masks, V weights/biases) on the gpsimd SWDGE queues.
            wq, wk, wo = [], [], []
            for p in range(PAIRS):
                tk = pp.tile([128, E], DT, tag=f"wk{p}", name=f"wk{p}")
                nc.scalar.dma_start(tk[:], d_wk[p * 128:(p + 1) * 128, :])
                wk.append(tk)
                tq = pp.tile([128, E], DT, tag=f"wq{p}", name=f"wq{p}")
                nc.scalar.dma_start(tq[:], d_wq[p * 128:(p + 1) * 128, :])
                wq.append(tq)
                to = pp.tile([128, E], DT, tag=f"wo{p}", name=f"wo{p}")
                wo.append(to)
            bqk = pp.tile([128, 2 * PAIRS], F32, tag="bqk", name="bqk")
            nc.scalar.dma_start(bqk[:], d_bqk[:, :])
            bq = [bqk[:, p:p + 1] for p in range(PAIRS)]
            bk = [bqk[:, PAIRS + p:PAIRS + p + 1] for p in range(PAIRS)]
            bo = []
            wv_all = pp.tile([128, VW * HPC * EC], DT, tag="wv", name="wv_all")
            nc.scalar.dma_start(wv_all[:], d_wv[:, :])
            wv = [wv_all[:, e * VW * HPC:(e + 1) * VW * HPC] for e in range(EC)]
            # xT in 8 batched transfers ([128, 3, 512] each): one dma_start
            # costs ~600ns of engine time, so 8 descriptors instead of 24
            # frees the sync engine ~10us earlier; half-quarter granularity
            # still lets the first projection chain start on e0-2.
            # sync-ring order matters (transfers run in order at wire speed):
            # xT quarter 0 first (first projections), then wv/bv/masks
            # (needed ~18-25us; the gpsimd SWDGE queue is ~80GB/s-slow and
            # the scalar ring's issue flow-control would hold them to ~18us),
            # then xT quarters 1-3 (needed at ~19/~40/~60us).
            d_xT3 = d_xT.rearrange("(e p) s -> p e s", p=128)
            xbig = [[None, None] for _ in range(4)]
            for quarter in range(4):
                for half in range(2):
                    xbig[quarter][half] = pp.tile(
                        [128, 3 * QB], DT, tag=f"xt{quarter}_{half}",
                        name=f"xt{quarter}_{half}")

            def _xtq_dma(quarter):
                hs = slice(quarter * QB, (quarter + 1) * QB)
                for half in range(2):
                    t = xbig[quarter][half]
                    nc.sync.dma_start(
                        t[:].rearrange("p (e s) -> p e s", s=QB),
                        d_xT3[:, 3 * half:3 * half + 3, hs])



            def xt(e, sb, c0=0, c1=QB):
                t = xbig[sb][e // 3]
                base = (e % 3) * QB
                return t[:, base + c0:base + c1]
            for p in range(PAIRS):
                nc.scalar.dma_start(wo[p][:], d_wo[p * 128:(p + 1) * 128, :])
            boall = pp.tile([128, EC], F32, tag="bo", name="boall")
            nc.scalar.dma_start(boall[:], d_bo[:, :])
            iden = pp.tile([128, 128], DT, tag="iden", name="iden")
            nc.scalar.dma_start(iden[:], d_iden[:, :])
            bo = [boall[:, e:e + 1] for e in range(EC)]
            bv = pp.tile([128, VW * HPC], F32, tag="bv")
            maskall = pp.tile([KB, 4 * 2 * KB], DT, tag="mask", name="maskall")
            masks = [maskall[:, o * 2 * KB:(o + 1) * 2 * KB] for o in range(4)]

            _xtq_dma(0)
            nc.sync.dma_start(bv[:], d_bv[:, :])
            nc.sync.dma_start(maskall[:], d_mask[:, :])
            for quarter in range(1, 4):
                _xtq_dma(quarter)

            kt = [[pp.tile([128, QB], DT, tag=f"kt{p}_{sb}", name=f"kt{p}_{sb}")
                   for sb in range(NQB)] for p in range(PAIRS)]
            qt = [[pp.tile([128, QB], DT, tag=f"qt{p}_{sb}", name=f"qt{p}_{sb}")
                   for sb in range(NQB)] for p in range(PAIRS)]
            vt = [pp.tile([128, VW * HPC], DT, tag=f"vt{s}", name=f"vt{s}") for s in range(SC)]

            Iden = mybir.ActivationFunctionType.Identity

            def _mk_chain():
                def chain(name, width, lhs_of_e, rhs_of_e, copy_out):
                    st = {}
                    def mk(e):
                        def step():
                            if e == 0:
                                st["ps"] = psA.tile(
                                    [128, width], F32, tag="misc", bufs=2,
                                    name=name)
                            nc.tensor.matmul(st["ps"][:],
                                             R(lhs_of_e(e)), R(rhs_of_e(e)),
                                             start=(e == 0), stop=(e == EC - 1))
                        return step
                    for e in range(EC):
                        yield mk(e)
                    yield lambda: copy_out(st["ps"])
                return chain

            def kq_pair_ops(sb, p, chain=None):
                # bias-add copies on DVE, keeping ACT free for the exp pipeline
                chain = chain or _mk_chain()
                kcopy = lambda ps, p=p, sb=sb: nc.vector.tensor_scalar_add(
                    kt[p][sb][:], ps[:], bk[p])
                qcopy = lambda ps, p=p, sb=sb: nc.vector.tensor_scalar_add(
                    qt[p][sb][:], ps[:], bq[p])
                yield from chain(
                    f"kps{p}_{sb}", QB,
                    lambda e, p=p: wk[p][:, e * 128:(e + 1) * 128],
                    lambda e, sb=sb: xt(e, sb), kcopy)
                yield from chain(
                    f"qps{p}_{sb}", QB,
                    lambda e, p=p: wq[p][:, e * 128:(e + 1) * 128],
                    lambda e, sb=sb: xt(e, sb), qcopy)

            def kq_ops(sb, chain=None):
                for p in range(PAIRS):
                    yield from kq_pair_ops(sb, p, chain)

            def v_ops(sb, chain=None):
                chain = chain or _mk_chain()
                for s in range(4 * sb, 4 * sb + 4):
                    yield from chain(
                        f"vps{s}", VW * HPC,
                        lambda e, sb=sb, s=s: xt(e, sb, (s % 4) * 128, (s % 4 + 1) * 128),
                        lambda e: wv[e],
                        lambda ps, s=s: nc.vector.tensor_add(
                            vt[s][:], ps[:], bv[:]))

            def make_normalize(qb, zpair):
                def normalize(head, zsb, bcast, unused=False):
                    # deferred: z * (1/denom), recip+broadcast already done.
                    # NB must stay on DVE: a gpsimd tensor op forces a ~16us
                    # dge_drain (SWDGE<->compute mode switch) on that engine.
                    p, sub = head // 2, head % 2
                    hsl = slice(sub * 64, sub * 64 + 64)
                    nc.vector.tensor_mul(zpair[p][hsl, :], zsb[:], bcast[:])
                return normalize

            def attention(qb, drain=None, late=None, last_pair_drain=None,
                          zpair_override=None, prereq=None, vt_gate=None):
                q0 = qb * QB
                qsl = slice(q0, q0 + QB)
                nkb = 4 * qb + 4
                # drain elements: zero-arg closures, or ("m", key) markers
                dq = list(drain) if drain is not None else []
                seen = set()
                iters = [PAIRS * max(nkb - 1, 1), 0]

                def _pop1():
                    el = dq.pop(0)
                    if isinstance(el, tuple):
                        seen.add(el[1])
                    else:
                        el()

                def drain_some():
                    if not dq:
                        return
                    n = max(1, -(-len(dq) // max(iters[0] - iters[1], 1)))
                    for _ in range(n):
                        if dq:
                            _pop1()
                    iters[1] += 1

                def drain_until(key):
                    # force-drain so a prerequisite chain is fully EMITTED
                    # before instructions that depend on it (emission order on
                    # an engine is execution order — a dep on a later
                    # instruction would deadlock)
                    while key not in seen and dq:
                        _pop1()
                zpair = zpair_override or [
                    zp.tile([128, QB], DT, tag=f"zp{p}", name=f"zp{p}_{qb}")
                    for p in range(PAIRS)]
                normalize = make_normalize(qb, zpair)
                pending = []
                for p in range(PAIRS):
                    if prereq and p in prereq:
                        drain_until(prereq[p])
                    zab = [psA.tile([VW, QB], F32, tag="z", bufs=2,
                                    name=f"zps{qb}_{2 * p + s}") for s in range(2)]

                    def qk(kb):
                        # both heads of the pair, concurrent via PE row groups.
                        # Diagonal-band blocks (kb >= 4qb) only need queries
                        # q >= (kb-4qb)*128: trim the streamed q range.
                        co = max(0, (kb - 4 * qb) * KB)
                        sps = psA.tile([KB, 2 * QB], F32, tag="s", bufs=2,
                                       name=f"sps{qb}_{p}_{kb}")
                        ktt = kt[p][kb // 4]
                        ksl = slice((kb % 4) * KB, (kb % 4 + 1) * KB)
                        nc.tensor.matmul(
                            sps[:, co:QB], R(ktt[0:64, ksl]), R(qt[p][qb][0:64, co:QB]),
                            start=True, stop=True, tile_position=(0, 0))
                        nc.tensor.matmul(
                            sps[:, QB + co:2 * QB], R(ktt[64:128, ksl]), R(qt[p][qb][64:128, co:QB]),
                            start=True, stop=True, tile_position=(64, 0))
                        return sps

                    def pv(kb, sps):
                        co = max(0, (kb - 4 * qb) * KB)
                        pt = wp.tile([KB, 2 * QB], DT, tag="p", bufs=6,
                                     name=f"pt{qb}_{p}_{kb}")
                        if co:
                            # strided [128, 2, QB-co] view covering both heads
                            sv = sps[:].rearrange("k (two q) -> k two q", two=2)[:, :, co:]
                            ptv = pt[:].rearrange("k (two q) -> k two q", two=2)[:, :, co:]
                            nc.scalar.activation(ptv, sv, Exp, scale=float(SCALE))
                        else:
                            nc.scalar.activation(pt[:], sps[:], Exp, scale=float(SCALE))
                        if kb >= 4 * qb:  # diagonal 128-col chunk: zero out k > q
                            o = kb - 4 * qb
                            ptd = pt[:].rearrange("k (two q) -> k two q", two=2)[:, :, co:co + KB]
                            mv = masks[o].rearrange("k (two q) -> k two q", two=2)
                            nc.vector.tensor_mul(ptd, ptd, mv)
                        for s in range(2):
                            nc.tensor.matmul(
                                zab[s][:, co:QB], R(vt[kb][:, (2 * p + s) * VW:(2 * p + s + 1) * VW]),
                                R(pt[:, s * QB + co:(s + 1) * QB]),
                                start=(kb == 0), stop=(kb == nkb - 1))

                    prev = qk(0)
                    for kb in range(1, nkb):
                        cur = qk(kb)
                        if vt_gate and kb - 1 == 4 * qb:
                            # band PV needs this quarter's vt chains emitted
                            drain_until(vt_gate)
                        pv(kb - 1, prev)
                        drain_some()
                        prev = cur
                        if kb == 2:
                            for args in pending:
                                normalize(*args)
                            pending = []
                            if p == PAIRS - 1 and last_pair_drain is not None:
                                dq.extend(last_pair_drain)
                    pv(nkb - 1, prev)
                    drain_some()

                    last = (qb == NQB - 1 and p == PAIRS - 1)
                    if last:
                        # tail: spread the readout chain across engines so the
                        # recip -> bcast -> mul critical path starts as early
                        # as possible (pass2 waits on the muls)
                        for s in range(2):
                            head = 2 * p + s
                            den = wp.tile([1, QB], F32, tag="den",
                                          name=f"den{qb}_{head}")
                            nc.scalar.activation(den[:], zab[s][64:65, :], Iden)
                            recipf = wp.tile([1, QB], F32, tag="recipf",
                                             name=f"recipf{qb}_{head}")
                            nc.vector.reciprocal_approx_fast(recipf[:], den[:])
                            zsb = wp.tile([64, QB], F32, tag="zc",
                                          name=f"zsb{qb}_{head}")
                            if s == 0:
                                nc.scalar.activation(zsb[:], zab[s][0:64, :], Iden)
                            else:
                                nc.vector.tensor_copy(zsb[:], zab[s][0:64, :])
                            bcast = wp.tile([64, QB], F32, tag="bcast",
                                            name=f"bcast{qb}_{head}")
                            nc.gpsimd.partition_broadcast(bcast[:], recipf[:])
                            pending.append((head, zsb, bcast, s == 0))
                    else:
                        # PSUM-freeing copies first (the next pair's PV waits
                        # on the zab banks), recip/broadcast after; in the
                        # PE-bound early quarters head 1's copies go to the
                        # then-idle ACT so the banks free ~2x sooner
                        zts = []
                        for s in range(2):
                            head = 2 * p + s
                            den = wp.tile([1, QB], F32, tag="den",
                                          name=f"den{qb}_{head}")
                            zsb = wp.tile([64, QB], F32, tag="zc",
                                          name=f"zsb{qb}_{head}")
                            if qb <= 1 and s == 1:
                                nc.scalar.activation(den[:], zab[s][64:65, :], Iden)
                                nc.scalar.activation(zsb[:], zab[s][0:64, :], Iden)
                            else:
                                nc.vector.tensor_copy(den[:], zab[s][64:65, :])
                                nc.vector.tensor_copy(zsb[:], zab[s][0:64, :])
                            zts.append((head, den, zsb))
                        for head, den, zsb in zts:
                            recipf = wp.tile([1, QB], F32, tag="recipf",
                                             name=f"recipf{qb}_{head}")
                            nc.vector.reciprocal_approx_fast(recipf[:], den[:])
                            bcast = wp.tile([64, QB], F32, tag="bcast",
                                            name=f"bcast{qb}_{head}")
                            nc.gpsimd.partition_broadcast(bcast[:], recipf[:])
                            pending.append((head, zsb, bcast, False))
                for args in pending:
                    normalize(*args)
                while dq:
                    _pop1()
                if late is not None:
                    for step in late:
                        step()
                return outproj_ops(qb, zpair)

            def outproj_split(qb, zpair):
                """qb=3 variant: p0+p1 partials run early (PE bubbles during
                the last pair), only the short p2 pass waits on the final
                normalize."""
                qsl = slice(qb * QB, (qb + 1) * QB)
                partial = [None] * EC

                def pass1():
                    for e in range(EC):
                        st = {}
                        def mk(e, p, st=st):
                            def step():
                                if p == 0:
                                    st["ps"] = psA.tile(
                                        [128, QB], F32, tag="misc", bufs=2,
                                        name=f"opsa{qb}_{e}")
                                nc.tensor.matmul(
                                    st["ps"][:], R(wo[p][:, e * 128:(e + 1) * 128]),
                                    R(zpair[p][:]), start=(p == 0), stop=(p == 1))
                            return step
                        yield mk(e, 0)
                        yield mk(e, 1)
                        def fin(e, st=st):
                            def step():
                                t = op.tile([128, QB], F32, tag=f"partial{e}",
                                            name=f"partial{qb}_{e}")
                                partial[e] = t
                                nc.vector.tensor_copy(t[:], st["ps"][:])
                            return step
                        yield fin(e)

                def pass2():
                    for e in range(EC):
                        st = {}
                        def mk(e, st=st):
                            def step():
                                st["ps"] = psA.tile([128, QB], F32, tag="misc",
                                                    bufs=2, name=f"opsb{qb}_{e}")
                                nc.tensor.matmul(
                                    st["ps"][:], R(wo[2][:, e * 128:(e + 1) * 128]),
                                    R(zpair[2][:]), start=True, stop=True)
                            return step
                        yield mk(e)
                        def fin(e, st=st):
                            def step():
                                osb = op.tile([128, QB], DT, tag="osb",
                                              name=f"osb{qb}_{e}")
                                nc.vector.scalar_tensor_tensor(
                                    osb[:], st["ps"][:], bo[e],
                                    partial[e][:],
                                    op0=mybir.AluOpType.add,
                                    op1=mybir.AluOpType.add)
                                # tail: split the final-quarter stores across
                                # both HWDGE rings (issue + wire in parallel)
                                deng = nc.sync if e % 2 == 0 else nc.scalar
                                deng.dma_start(
                                    d_out[e * 128:(e + 1) * 128, qsl], osb[:])
                            return step
                        yield fin(e)
                return pass1, pass2

            def outproj_ops(qb, zpair):
                qsl = slice(qb * QB, (qb + 1) * QB)
                for e in range(EC):
                    st = {}
                    def mk(e, p):
                        def step():
                            if p == 0:
                                st["ps"] = psA.tile([128, QB], F32, tag="misc",
                                                    bufs=2, name=f"ops{qb}_{e}")
                            nc.tensor.matmul(
                                st["ps"][:], R(wo[p][:, e * 128:(e + 1) * 128]),
                                R(zpair[p][:]),
                                start=(p == 0), stop=(p == PAIRS - 1))
                        return step
                    for p in range(PAIRS):
                        yield mk(e, p)
                    def fin(e):
                        def step():
                            osb = op.tile([128, QB], DT, tag="osb",
                                          name=f"osb{qb}_{e}")
                            nc.vector.tensor_scalar_add(osb[:], st["ps"][:],
                                                        bo[e])
                            nc.sync.dma_start(d_out[e * 128:(e + 1) * 128, qsl],
                                              osb[:])
                        return step
                    yield fin(e)

            for step in kq_ops(0):
                step()
            for step in v_ops(0):
                step()
            carry = []
            for qb in range(NQB):
                if qb == 0:
                    drain = list(carry) + list(kq_ops(1))
                    oops = attention(qb, drain=iter(drain), late=v_ops(1))
                    carry = list(oops)
                elif qb == 1:
                    drain = list(carry) + list(kq_ops(2))
                    oops = attention(qb, drain=iter(drain), late=v_ops(2))
                    carry = list(oops)
                elif qb == 2:
                    # quarter balance: attention(2) is PE-drain-saturated while
                    # attention(3) is ACT-bound with idle PE — keep only the
                    # next quarter's pair-0 K/Q chains here and push pairs 1-2
                    # plus the V chains into attention(3)'s bubbles (gated by
                    # markers so chains are emitted before their consumers)
                    drain = list(carry) + list(kq_pair_ops(3, 0))
                    oops = attention(qb, drain=iter(drain))
                    carry = list(oops)
                else:
                    zpair_last = [zp.tile([128, QB], DT, tag=f"zp{p}",
                                          name=f"zpL{p}") for p in range(PAIRS)]
                    pass1, pass2 = outproj_split(qb, zpair_last)
                    drain = (list(v_ops(qb)) + [("m", "vt3")]
                             + list(kq_pair_ops(qb, 1)) + [("m", "kq31")]
                             + list(kq_pair_ops(qb, 2)) + [("m", "kq32")]
                             + list(carry))
                    attention(qb, drain=iter(drain),
                              last_pair_drain=pass1(),
                              zpair_override=zpair_last,
                              prereq={1: "kq31", 2: "kq32"},
                              vt_gate="vt3")
                    for step in pass2():
                        step()

    nc.compile()
    return nc


def _get_nc():
    if _g["nc"] is None:
        _g["nc"] = _build()
    return _g["nc"]


def _make_in_maps(inputs):
    x = np.asarray(inputs["normalized_resid_pre"], dtype=np.float32)
    W_Q = np.asarray(inputs["W_Q"], dtype=np.float32)
    W_K = np.asarray(inputs["W_K"], dtype=np.float32)
    W_V = np.asarray(inputs["W_V"], dtype=np.float32)
    W_O = np.asarray(inputs["W_O"], dtype=np.float32)
    b_Q = np.asarray(inputs["b_Q"], dtype=np.float32)
    b_K = np.asarray(inputs["b_K"], dtype=np.float32)
    b_V = np.asarray(inputs["b_V"], dtype=np.float32)
    b_O = np.asarray(inputs["b_O"], dtype=np.float32)
    dt = _np_dt()

    # 0/1 keep-masks for the diagonal 128-col chunk of each band offset,
    # [4*128, 2*128]; both 128-col halves carry the same triangular pattern
    # (they hold the two heads of a pair). Same for all 4 offsets, but kept
    # per-offset so each band block multiplies its own tile.
    # [128, 4 offsets x 2 heads x 128]: same triangular pattern per offset
    mask = np.zeros((KB, 4 * 2 * KB), dtype=dt)
    for o in range(4):
        for dk in range(KB):
            for half in range(2):
                base = o * 2 * KB + half * KB
                mask[dk, base + dk: base + KB] = 1.0
    in_maps = []
    for c in range(8):
        b = c // 2
        hs = (c % 2) * HPC
        heads = list(range(hs, hs + HPC))
        def pack(w):
            # [E, C] -> [128, EC*C] with column block e holding rows e*128..
            C = w.shape[1]
            return np.ascontiguousarray(
                w.reshape(EC, 128, C).transpose(1, 0, 2).reshape(128, EC * C))

        wq = np.concatenate(
            [pack(np.concatenate([W_Q[heads[2 * p]], W_Q[heads[2 * p + 1]]], axis=1))
             for p in range(PAIRS)], axis=0)             # [3*128, 768]
        wk = np.concatenate(
            [pack(np.concatenate([W_K[heads[2 * p]], W_K[heads[2 * p + 1]]], axis=1))
             for p in range(PAIRS)], axis=0)
        wv = np.zeros((E, VW * HPC), dtype=np.float32)
        bv = np.zeros((128, VW * HPC), dtype=np.float32)
        for h in range(HPC):
            wv[:, h * VW: h * VW + H] = W_V[heads[h]]
            bv[:, h * VW: h * VW + H] = b_V[heads[h]][None, :]
            bv[:, h * VW + H] = 1.0
        wv = pack(wv)                                    # [128, 6*390]
        wo = np.concatenate(
            [np.concatenate([W_O[heads[2 * p]], W_O[heads[2 * p + 1]]], axis=0)
             for p in range(PAIRS)], axis=0)             # [3*128, 768]
        # bqk [128, 6]: cols 0..2 = bq per pair, cols 3..5 = bk per pair
        bqk = np.zeros((128, 2 * PAIRS), dtype=np.float32)
        for p in range(PAIRS):
            bqk[:, p] = np.concatenate(
                [b_Q[heads[2 * p]], b_Q[heads[2 * p + 1]]])
            bqk[:, PAIRS + p] = np.concatenate(
                [b_K[heads[2 * p]], b_K[heads[2 * p + 1]]])
        # bo [128, 6]: col e = rows e*128..(e+1)*128 of b_O/2
        bo2 = np.ascontiguousarray(
            (b_O / 2.0).reshape(EC, 128).T)
        in_maps.append({
            "xT": np.ascontiguousarray(x[b].T).astype(dt),
            "wq": wq.astype(dt), "wk": wk.astype(dt),
            "wv": wv.astype(dt), "wo": wo.astype(dt),
            "bqk": bqk, "bv": bv, "bo": bo2,
            "mask": mask, "iden": np.eye(128, dtype=dt),
        })
    return in_maps


def _gather(results):
    out = np.empty((B, S, E), dtype=np.float32)
    for b in range(B):
        acc = results[2 * b]["outT"].astype(np.float32) + \
              results[2 * b + 1]["outT"].astype(np.float32)
        out[b] = acc.T
    return out


def run(inputs, trace=False):
    """Returns (output, BassKernelResults)."""
    from concourse.bass_utils import run_bass_kernel_spmd

    if trace:
        _install_ntff_shim()
    nc = _get_nc()
    in_maps = _make_in_maps(inputs)
    res = run_bass_kernel_spmd(nc, in_maps, core_ids=list(range(8)), trace=trace)
    return _gather(res.results), res


def kernel(**inputs):
    out, _ = run(inputs, trace=False)
    return out


def _install_ntff_shim():
    """The agent image's antenv lacks axon_hooks; recreate it so
    run_bass_kernel_spmd(trace=True) can capture NTFF profiles."""
    import types, ctypes, contextlib

    if "antenv.axon_hooks" in sys.modules:
        return
    so_path = "/opt/axon/libaxon_pjrt.so"
    try:
        lib = ctypes.CDLL(so_path)
        lib.axon_start_nrt_profile.argtypes = [ctypes.POINTER(ctypes.c_int64),
                                              ctypes.c_size_t]
        lib.axon_start_nrt_profile.restype = ctypes.c_int64
        lib.axon_stop_nrt_profile.argtypes = [ctypes.c_char_p]
        lib.axon_stop_nrt_profile.restype = ctypes.c_int64
    except (OSError, AttributeError):
        return

    @contextlib.contextmanager
    def _hook(output_dir, device_ids):
        import jax

        jax.devices()
        if device_ids:
            ids = (ctypes.c_int64 * len(device_ids))(*device_ids)
            rc = lib.axon_start_nrt_profile(ids, len(device_ids))
        else:
            rc = lib.axon_start_nrt_profile(None, 0)
        if rc != 0:
            raise RuntimeError(f"axon_start_nrt_profile rc={rc}")
        try:
            yield
        finally:
            n = lib.axon_stop_nrt_profile(str(output_dir).encode())
            print(f"ntff profile: {n} file(s) -> {output_dir}", file=sys.stderr)

    mod = types.ModuleType("antenv.axon_hooks")
    mod.get_axon_ntff_profile_hook = lambda: _hook
    sys.modules["antenv.axon_hooks"] = mod
    # avoid S3 upload attempts from the trace post-processing
    from concourse import bass_utils as bu

    bu.upload_artifacts = lambda tmpdir: f"local:{tmpdir}"

